# revision 5
# baseline (speedup 1.0000x reference)
"""TRN2 Bass/Tile kernel: GNN message-passing self-attention (BertSelfAttention).

Math (per node n, head h):
    q = h @ Wq.T + bq                                  (own node)
    k/v = (h @ W{k,v}.T + b{k,v})[neighbor_idx]        (gathered rows)
    scores = q.k / sqrt(dh) + mask[neighbor_idx]
    ctx = softmax(scores) @ v

Strategy (8 cores, SPMD):
  - Shard destination nodes across cores (2500/core, padded to 2560).
  - Each core computes fused [K|V|Q] projections for ONLY its own 2560 nodes
    (20 PE subtiles x 4 matmuls, fused 768-wide weight), keeps Q on-chip,
    stores its K|V shard to DRAM with ONE contiguous SWDGE DMA (table rows
    host-permuted so the store needs no scatter), then an AllGather collective
    replicates the full 20480-row K|V table into every core's DRAM.
  - Per 128-node tile, 16 SWDGE indirect DMAs gather the 2048 neighbor K|V
    rows (1KB each); per-edge attention on DVE with broadcast access
    patterns; exp on ACT. 1/sqrt(dh) folded into Wq; softmax normalization
    folded into the final context scale.
  - ALL data movement uses the SWDGE (gpsimd) DMA path: on this runtime
    HWDGE (nc.sync) DMAs cost ~50us per call while SWDGE calls are ~1us,
    and PE/PSUM-touching instructions carry a ~40-70us dispatch cost -- the
    shard+AllGather build exists precisely to cut matmul/copy count 8x vs.
    building the full table per core. (Measured via in-NEFF repeat
    differencing; see probe_* scripts.)
  - bf16 table/products with fp32 accumulation: rel err ~3.4e-3 end to end.
"""

import math

import ml_dtypes
import numpy as np

import concourse.bass as bass
import concourse.mybir as mybir
import concourse.tile as tile
from concourse import bacc
from concourse.bass_utils import run_bass_kernel_spmd

# Problem constants (fixed by the harness contract).
N_CORES = 8
N_NODES = 20000
H = 256  # hidden size
D = 16  # neighbors per node
NH = 8  # heads
DH = 32  # head dim
P = 128  # partitions
KVW = 2 * H  # fused K|V row width (512)
W3 = KVW + H  # fused K|V|Q width (768)

LOCAL = N_NODES // N_CORES  # 2500
NT = (LOCAL + P - 1) // P  # 20 node tiles per core
LPAD = NT * P  # 2560
NPAD = LPAD * N_CORES  # 20480 table rows

F32 = mybir.dt.float32
BF16 = mybir.dt.bfloat16
I32 = mybir.dt.int32
BF = ml_dtypes.bfloat16


def build_program(with_bias=False, repeat2=1, repeat_all=1):
    """Build the SPMD single-core Bass program (identical across cores)."""
    nc = bacc.Bacc("TRN2", target_bir_lowering=False, debug=False)

    hTl = nc.dram_tensor("hTl", [H, LPAD], BF16, kind="ExternalInput")
    wkvq = nc.dram_tensor("wkvq", [H, W3], BF16, kind="ExternalInput")
    idxg = nc.dram_tensor("idxg", [P, NT, D], I32, kind="ExternalInput")
    maskg = nc.dram_tensor("maskg", [P, NT, D], F32, kind="ExternalInput")
    if with_bias:
        bkvq = nc.dram_tensor("bkvq", [1, W3], BF16, kind="ExternalInput")
    out = nc.dram_tensor("out", [LPAD, H], F32, kind="ExternalOutput")
    shard = nc.dram_tensor("shard", [LPAD, KVW], BF16)
    kvtab = nc.dram_tensor("kvtab", [NPAD, KVW], BF16, addr_space="Shared")

    with tile.TileContext(nc) as tc:
        with (
            tc.tile_pool(name="persist", bufs=1) as wpool,
            tc.tile_pool(name="psum", bufs=4, space="PSUM") as pspool,
            tc.tile_pool(name="gath", bufs=3) as gpool,
            tc.tile_pool(name="prod", bufs=2) as prodpool,
            tc.tile_pool(name="small", bufs=3) as smpool,
            tc.tile_pool(name="ctx", bufs=2) as ctxpool,
        ):
            # ---- persistent loads (all SWDGE) ----
            wkvq_t = wpool.tile([P, 2, W3], BF16)
            nc.gpsimd.dma_start(wkvq_t[:, 0, :], wkvq[0:P, :])
            nc.gpsimd.dma_start(wkvq_t[:, 1, :], wkvq[P:H, :])
            idx_all = wpool.tile([P, NT, D], I32)
            nc.gpsimd.dma_start(idx_all[:], idxg[:])
            mask_all = wpool.tile([P, NT, D], F32)
            nc.gpsimd.dma_start(mask_all[:], maskg[:])
            hTl_t = wpool.tile([P, 2, LPAD], BF16)
            nc.gpsimd.dma_start(hTl_t[:, 0, :], hTl[0:P, :])
            nc.gpsimd.dma_start(hTl_t[:, 1, :], hTl[P:H, :])
            if with_bias:
                ones_t = wpool.tile([1, P], BF16)
                nc.vector.memset(ones_t[:], 1.0)
                bkvq_t = wpool.tile([1, W3], BF16)
                nc.gpsimd.dma_start(bkvq_t[:], bkvq[:])
            # K|V|Q stage for own nodes: [:, t, 0:512]=K|V, [:, t, 512:768]=Q
            stage = wpool.tile([P, NT, W3], BF16)

            for _rep in range(repeat_all):
                # ---- phase 1: fused K|V|Q for own 2560 nodes ----
                for t in range(NT):
                    ps = pspool.tile([P, 1024], F32, tag="ps")
                    for lo, hi in ((0, KVW), (KVW, W3)):
                        nc.tensor.matmul(
                            ps[:, lo:hi],
                            hTl_t[:, 0, t * P : (t + 1) * P],
                            wkvq_t[:, 0, lo:hi],
                            start=True,
                            stop=False,
                        )
                        nc.tensor.matmul(
                            ps[:, lo:hi],
                            hTl_t[:, 1, t * P : (t + 1) * P],
                            wkvq_t[:, 1, lo:hi],
                            start=False,
                            stop=not with_bias,
                        )
                        if with_bias:
                            nc.tensor.matmul(
                                ps[:, lo:hi],
                                ones_t[:],
                                bkvq_t[:, lo:hi],
                                start=False,
                                stop=True,
                            )
                    nc.scalar.copy(stage[:, t, :], ps[:, 0:W3])

                # ---- one contiguous shard store + AllGather ----
                # shard row r = p*NT + t holds local node m = t*128 + p
                nc.gpsimd.dma_start(
                    shard[:].rearrange("(p t) e -> p t e", p=P),
                    stage[:, :, 0:KVW],
                )
                nc.gpsimd.collective_compute(
                    "AllGather",
                    mybir.AluOpType.bypass,
                    replica_groups=[list(range(N_CORES))],
                    ins=[shard[:]],
                    outs=[kvtab[:]],
                )

                # ---- phase 2: gather + attention per 128-node tile ----
                for t in [tt for _ in range(repeat2) for tt in range(NT)]:
                    kvg = gpool.tile([P, D, KVW], BF16, tag="kvg")
                    for d in range(D):
                        nc.gpsimd.indirect_dma_start(
                            out=kvg[:, d, :],
                            out_offset=None,
                            in_=kvtab[:],
                            in_offset=bass.IndirectOffsetOnAxis(
                                ap=idx_all[:, t, d : d + 1], axis=0
                            ),
                        )
                    k_view = kvg[:, :, 0:H].rearrange(
                        "p d (nh dh) -> p d nh dh", nh=NH
                    )
                    v_view = kvg[:, :, H:KVW].rearrange(
                        "p d (nh dh) -> p d nh dh", nh=NH
                    )
                    q_view = (
                        stage[:, t, KVW:W3]
                        .rearrange("p (nh dh) -> p nh dh", nh=NH)
                        .unsqueeze(1)
                        .broadcast_to([P, D, NH, DH])
                    )

                    # scores[p, d, h] = sum_c q*k (1/sqrt(dh) pre-folded)
                    mk = prodpool.tile([P, D, NH, DH], BF16, tag="mk")
                    nc.vector.tensor_tensor(
                        out=mk[:], in0=k_view, in1=q_view, op=mybir.AluOpType.mult
                    )
                    scores = smpool.tile([P, D, NH], F32, tag="scores")
                    nc.vector.tensor_reduce(
                        out=scores[:],
                        in_=mk[:],
                        axis=mybir.AxisListType.X,
                        op=mybir.AluOpType.add,
                    )
                    scores_m = smpool.tile([P, D, NH], F32, tag="scores_m")
                    nc.vector.tensor_tensor(
                        out=scores_m[:],
                        in0=scores[:],
                        in1=mask_all[:, t, :].unsqueeze(2).broadcast_to([P, D, NH]),
                        op=mybir.AluOpType.add,
                    )
                    # softmax over d (unnormalized; 1/sum folded into ctx scale)
                    smax = smpool.tile([P, NH], F32, tag="smax")
                    nc.vector.tensor_reduce(
                        out=smax[:],
                        in_=scores_m[:].transpose([0, 2, 1]),
                        axis=mybir.AxisListType.X,
                        op=mybir.AluOpType.max,
                    )
                    s2 = smpool.tile([P, D, NH], F32, tag="s2")
                    nc.vector.tensor_tensor(
                        out=s2[:],
                        in0=scores_m[:],
                        in1=smax[:].unsqueeze(1).broadcast_to([P, D, NH]),
                        op=mybir.AluOpType.subtract,
                    )
                    pexp = smpool.tile([P, D, NH], BF16, tag="pexp")
                    nc.scalar.activation(
                        pexp[:], s2[:], mybir.ActivationFunctionType.Exp
                    )
                    sumexp = smpool.tile([P, NH], F32, tag="sumexp")
                    nc.vector.tensor_reduce(
                        out=sumexp[:],
                        in_=pexp[:].transpose([0, 2, 1]),
                        axis=mybir.AxisListType.X,
                        op=mybir.AluOpType.add,
                    )
                    rsum = smpool.tile([P, NH], F32, tag="rsum")
                    nc.vector.reciprocal(rsum[:], sumexp[:])

                    # ctx[p, h, c] = (sum_d pexp * v) * rsum
                    mv = prodpool.tile([P, D, NH, DH], BF16, tag="mv")
                    nc.vector.tensor_tensor(
                        out=mv[:],
                        in0=v_view,
                        in1=pexp[:].unsqueeze(3).broadcast_to([P, D, NH, DH]),
                        op=mybir.AluOpType.mult,
                    )
                    ctx_un = ctxpool.tile([P, NH, DH], F32, tag="ctx_un")
                    nc.vector.tensor_reduce(
                        out=ctx_un[:],
                        in_=mv[:].transpose([0, 2, 3, 1]),
                        axis=mybir.AxisListType.X,
                        op=mybir.AluOpType.add,
                    )
                    ctx_f = ctxpool.tile([P, NH, DH], F32, tag="ctx_f")
                    nc.vector.tensor_tensor(
                        out=ctx_f[:],
                        in0=ctx_un[:],
                        in1=rsum[:].unsqueeze(2).broadcast_to([P, NH, DH]),
                        op=mybir.AluOpType.mult,
                    )
                    nc.gpsimd.dma_start(
                        out[t * P : (t + 1) * P, :],
                        ctx_f[:].rearrange("p nh dh -> p (nh dh)"),
                    )

    nc.finalize()
    return nc


def _row_of_node():
    """Global node id -> K|V table row (after per-core shard permutation).

    Core c's shard occupies rows [c*2560, (c+1)*2560); within it, local node
    m = t*128 + p sits at row p*NT + t (the contiguous-store layout).
    """
    n = np.arange(N_NODES, dtype=np.int64)
    c = n // LOCAL
    m = n - c * LOCAL
    return (c * LPAD + (m % P) * NT + (m // P)).astype(np.int32)


def prepare_inputs(
    h, attention_mask, neighbor_idx, Wq, bq, Wk, bk, Wv, bv,
):
    """Host-side sharding / layout prep. Returns (in_maps, with_bias)."""
    scale = np.float32(1.0 / math.sqrt(DH))

    h = np.asarray(h, dtype=np.float32)
    attention_mask = np.asarray(attention_mask, dtype=np.float32)
    neighbor_idx = np.asarray(neighbor_idx)
    Wq = np.asarray(Wq, dtype=np.float32)
    Wk = np.asarray(Wk, dtype=np.float32)
    Wv = np.asarray(Wv, dtype=np.float32)
    bq = np.asarray(bq, dtype=np.float32)
    bk = np.asarray(bk, dtype=np.float32)
    bv = np.asarray(bv, dtype=np.float32)

    with_bias = bool(np.any(bq) or np.any(bk) or np.any(bv))

    wkvq = np.ascontiguousarray(
        np.concatenate([Wk.T, Wv.T, Wq.T * scale], axis=1)
    ).astype(BF)  # [256, 768]
    bkvq = np.concatenate([bk, bv, bq * scale])[None, :].astype(BF)
    rowmap = _row_of_node()

    in_maps = []
    for c in range(N_CORES):
        lo = c * LOCAL
        hTl = np.zeros((H, LPAD), dtype=BF)
        hTl[:, :LOCAL] = h[lo : lo + LOCAL].T.astype(BF)

        nb = np.zeros((LPAD, D), dtype=np.int64)
        nb[:LOCAL] = neighbor_idx[lo : lo + LOCAL]
        # tile layout: idxg[p, t, d] = table_row(nb[t*128+p, d])
        idxc = np.ascontiguousarray(
            rowmap[nb].reshape(NT, P, D).transpose(1, 0, 2)
        ).astype(np.int32)
        mg = np.ascontiguousarray(
            attention_mask[nb].reshape(NT, P, D).transpose(1, 0, 2)
        ).astype(np.float32)

        m = dict(hTl=hTl, wkvq=wkvq, idxg=idxc, maskg=mg)
        if with_bias:
            m["bkvq"] = bkvq
        in_maps.append(m)
    return in_maps, with_bias


_PROGRAM_CACHE = {}


def _get_program(with_bias):
    if with_bias not in _PROGRAM_CACHE:
        _PROGRAM_CACHE[with_bias] = build_program(with_bias=with_bias)
    return _PROGRAM_CACHE[with_bias]


def kernel(h, attention_mask, neighbor_idx, Wq, bq, Wk, bk, Wv, bv, **run_kwargs):
    in_maps, with_bias = prepare_inputs(
        h, attention_mask, neighbor_idx, Wq, bq, Wk, bk, Wv, bv
    )
    nc = _get_program(with_bias)
    res = run_bass_kernel_spmd(nc, in_maps, list(range(N_CORES)), **run_kwargs)
    out = np.concatenate(
        [np.asarray(res.results[c]["out"])[:LOCAL] for c in range(N_CORES)], axis=0
    )
    result = out.astype(np.float32)
    if run_kwargs:
        return result, res
    return result
